# revision 27
# baseline (speedup 1.0000x reference)
"""Trainium2 Bass kernel for CustomFullyConnectedLayerGoogleTopK.

Math (from the reference, with IN_F == OUT_F == TOTAL_PERM == DIAG_LEN == 4096):
    a_topk = clip(K * softmax(alpha), 0, 1)                    # K = 3687
    Vs     = V * a_topk[:, None]                               # [4096, 4096]
    W[r,c] = Vs[(r - c) % 4096, c]   (scatter has no collisions)
    out    = x @ W.T                                           # [8192, 4096]

Device strategy: data-parallel over batch (8 cores x 1024 rows). The weight
W.T[c, r] = VsT[c, (r - c) % 4096] where VsT = Vs.T. Storing the doubled
array W2 = concat(VsT, VsT, axis=1) [4096, 8192] makes every [128, ntile]
tile of W.T a single affine access pattern: element (p, j) of the tile for
(k, n) lives at W2 linear offset (4096 + n*ntile) + p*8191 + k*128*8191 + j.
So the whole matmul streams with plain DMAs - no gather, no transpose.

Precision/speed hybrid: the contraction over 32 k-tiles is split into
(32 - q) bf16 k-tiles (1 cycle/row on the PE) and q fp8(e4m3) k-tiles
executed as q/2 DoubleRow matmuls (2 k-tiles per instruction at 1
cycle/row -> 2x throughput, verified on hw). The PE stream shrinks by
q/64. W is pre-scaled by 2^s and x by 2^-s for the fp8 tiles so both
operands sit in e4m3's normal range; the product scale is exactly 1 so
fp8 and bf16 partials accumulate in the same PSUM bank. All bf16
matmuls of an n-group run first (8 open PSUM groups), then one fp8
DoubleRow pass closes them - the PE pays only 2 dtype switches per
n-group instead of 2 per (m, n) tile.

Measured (core 0, NTFF profile), q=8 s=4: 403.4 us HW exec (was 458.8
us all-bf16), absmax-rel err 1.54e-2, L2-rel 1.84e-2 (both < 2e-2
gate; exactly matching the numpy emulation of the same quantization).
q=10 would break L2 (2.05e-2): don't raise q. fp16 as base dtype was
tried and is 20% SLOWER than bf16 on the PE when the stationary tensor
actually changes per instruction (259ns vs 216ns per 512-col matmul).
The remaining ~16us over the 387us PE floor: ~7.5us framework
preamble, 3.4us clock-warmup matmuls overlapping the first DMAs, ~6us
of environment-periodic PE stalls (one 380ns stall every 10.8us, also
present in the all-bf16 baseline), ~5us tail drain + postamble.

Each core: out_slice[1024, 4096] = xT_slice.T @ W.T via PE matmuls,
lhsT = xT tile [128c, 128b] (stationary), rhs = W.T tile [128c, ntile r].
"""

import os

import numpy as np
import ml_dtypes

B = 8192  # batch
F = 4096  # in_features == out_features == total_perm == diag_len
NCORES = 8
BS = B // NCORES  # batch rows per core
KTOPK = 3687  # ceil((1 - 0.1) * F * F / F)

Q = int(os.environ.get("GTOPK_Q", "8"))  # fp8 k-tiles (even, 0..32)
BASE = os.environ.get("GTOPK_BASE", "bf16")  # bf16 | fp16 base dtype
S = int(os.environ.get("GTOPK_S", "4"))  # fp8 scale exponent

_NC_CACHE = {}
_LAST_RESULTS = None  # stashed BassKernelResults for test harness introspection


def _build_nc(q, base, f=F, bs=BS):
    import concourse.bass as bass
    import concourse.tile as tile
    from concourse import bacc, mybir

    in16 = mybir.dt.bfloat16 if base == "bf16" else mybir.dt.float16
    in8 = mybir.dt.float8e4
    n_tile = 512

    k_tiles = f // 128  # 32
    kb = k_tiles - q  # fp16 k-tiles
    qp = q // 2  # fp8 DoubleRow pairs
    m_tiles = bs // 128
    n_tiles = f // n_tile
    w2w = 2 * f  # doubled width
    r0 = kb * 128  # first w2 row needed by the fp8 staircases

    # rows of the fp16 doubled array actually touched by its staircases
    max_off16 = f + (n_tiles - 1) * n_tile + ((kb - 1) * 128 + 127) * (w2w - 1) + (
        n_tile - 1
    )
    rows16 = max_off16 // w2w + 1 if kb > 0 else 0

    nc = bacc.Bacc(None, target_bir_lowering=False, debug=False)
    xt = nc.dram_tensor("xt", [kb * 128, bs], in16, kind="ExternalInput")
    w2 = nc.dram_tensor("w2", [rows16, w2w], in16, kind="ExternalInput")
    if q:
        x8t = nc.dram_tensor("x8t", [q * 128, bs], in8, kind="ExternalInput")
        w28 = nc.dram_tensor("w28", [q * 128, w2w], in8, kind="ExternalInput")
    out = nc.dram_tensor("out", [bs, f], mybir.dt.float32, kind="ExternalOutput")

    def xt_src(k):  # [128, bs] tile k of x.T slice
        return bass.AP(xt, k * 128 * bs, [[bs, 128], [1, bs]])

    def x8_src(j):  # [128, 2, bs] fp8 pair tile j (k-tiles kb+2j, kb+2j+1)
        return bass.AP(x8t, 256 * j * bs, [[bs, 128], [128 * bs, 2], [1, bs]])

    def wt_src(n, k):  # staircase [128, n_tile] tile of W.T (fp16)
        return bass.AP(
            w2, f + n * n_tile + k * 128 * (w2w - 1), [[w2w - 1, 128], [1, n_tile]]
        )

    def wt16_pair_src(n, k2):  # [128, 2, n_tile] fp16 pair (k-tiles 2k2, 2k2+1)
        return bass.AP(
            w2,
            f + n * n_tile + k2 * 256 * (w2w - 1),
            [[w2w - 1, 128], [128 * (w2w - 1), 2], [1, n_tile]],
        )

    def wt8_src(n, j):  # [128, 2, n_tile] fp8 DR pair (k-tiles kb+2j, kb+2j+1)
        return bass.AP(
            w28,
            f + n * n_tile + (kb + 2 * j) * 128 * (w2w - 1) - r0 * w2w,
            [[w2w - 1, 128], [128 * (w2w - 1), 2], [1, n_tile]],
        )

    DR = mybir.MatmulPerfMode.DoubleRow

    with tile.TileContext(nc) as tc:
        with (
            tc.tile_pool(name="xpool", bufs=kb + 1) as xpool,
            tc.tile_pool(name="x8pool", bufs=max(qp, 1)) as x8pool,
            tc.tile_pool(name="wpool", bufs=3 * max(kb // 2, 1)) as wpool,
            tc.tile_pool(name="w8pool", bufs=3 * max(qp, 1)) as w8pool,
            tc.tile_pool(name="opool", bufs=6) as opool,
            tc.tile_pool(name="ppool", bufs=8, space="PSUM") as ppool,
        ):
            # HAM warmup: N=128 matmuls on scratch fill the PE-idle window
            # between the framework start barrier and first data arrival, so
            # real matmuls start at the warm 2.4 GHz clock.
            warm = xpool.tile([128, 128], in16, name="warm", tag="warm", bufs=1)
            # gpsimd's queue clears the framework preamble before vector's,
            # so the warmup (and with it the real stream) starts earlier.
            nc.gpsimd.memset(warm[:], 0)
            ps_w = ppool.tile([128, n_tile], mybir.dt.float32, name="ps_w", tag="ps")
            # 32 x 128-col warmup matmuls ~= 3.4us at the mid p-state clock:
            # ends right when the first (x, w) tiles land (~10.9us). Fewer
            # warmups leave a PE idle gap before the data, which resets the
            # p-state and costs ~4.5us in re-ramp (measured).
            for _ in range(32):
                nc.tensor.matmul(
                    ps_w[:, :128], warm[:], warm[:],
                    start=True, stop=True, skip_group_check=True,
                )

            # x.T slice cached in SBUF as separate tiles so the scheduler can
            # start matmuls as soon as individual tiles land. xt[0] is split
            # so the very first (xt, wt) pair is only small. Interleave x/w
            # DMAs for n=0 so pairs arrive together.
            xts = []
            wt0 = []
            split0 = bs > 256
            if split0:
                # NOTE: issuing these first DMAs from the gpsimd queue was
                # tried and is ~4.5us WORSE: the transfers complete later
                # (~12.6us vs ~10.9us) and the resulting PE idle gap after
                # the warmup resets the clock p-state.
                # All input DMAs go through the sync queue: it is the only
                # queue whose kernel-section triggers land data early
                # (scalar/gpsimd triggers were measured to complete ~2-3us
                # later, leaving a PE idle gap after warmup that resets the
                # clock p-state at a ~4.5us cost).
                x0a = xpool.tile([128, 256], in16, name="xt0a", tag="xt")
                nc.sync.dma_start(
                    out=x0a[:], in_=bass.AP(xt, 0, [[bs, 128], [1, 256]])
                )
                w0 = wpool.tile([128, n_tile], in16, name="wt0_0", tag="wt")
                nc.sync.dma_start(out=w0[:], in_=wt_src(0, 0))
                x0b = xpool.tile([128, bs - 256], in16, name="xt0b", tag="xt")
                nc.sync.dma_start(
                    out=x0b[:], in_=bass.AP(xt, 256, [[bs, 128], [1, bs - 256]])
                )
                xts.append((x0a, x0b))
                wt0.append(w0)
            else:
                x0 = xpool.tile([128, bs], in16, name="xt0", tag="xt")
                nc.sync.dma_start(out=x0[:], in_=xt_src(0))
                w0 = wpool.tile([128, n_tile], in16, name="wt0_0", tag="wt")
                nc.sync.dma_start(out=w0[:], in_=wt_src(0, 0))
                xts.append(x0)
                wt0.append(w0)
            for k in range(1, kb):
                xk = xpool.tile([128, bs], in16, name=f"xt{k}", tag="xt")
                nc.sync.dma_start(out=xk[:], in_=xt_src(k))
                wk = wpool.tile([128, n_tile], in16, name=f"wt0_{k}", tag="wt")
                nc.sync.dma_start(out=wk[:], in_=wt_src(0, k))
                wt0.append(wk)
                xts.append(xk)
            x8s = []
            wt8_0 = []
            for j in range(qp):
                xj = x8pool.tile([128, 2, bs], in8, name=f"x8_{j}", tag="x8")
                nc.sync.dma_start(out=xj[:], in_=x8_src(j))
                wj = w8pool.tile([128, 2, n_tile], in8, name=f"wt8_0_{j}", tag="w8")
                nc.sync.dma_start(out=wj[:], in_=wt8_src(0, j))
                x8s.append(xj)
                wt8_0.append(wj)

            def xsl(k, m):  # lhsT block [128, 128] for (k-tile, m-tile)
                t = xts[k]
                if isinstance(t, tuple):
                    a, b = t
                    if m < 2:
                        return a[:, m * 128 : (m + 1) * 128]
                    return b[:, (m - 2) * 128 : (m - 1) * 128]
                return t[:, m * 128 : (m + 1) * 128]

            def x8sl(j, m):  # DR lhsT block [128, 2, 128]
                return x8s[j][:, :, m * 128 : (m + 1) * 128]

            wts = wt0
            wt8s = wt8_0
            for n in range(n_tiles):
                # prefetch next n's weight tiles (2 k-tiles per DMA: halves
                # the per-DMA issue load on the sync sequencer)
                if n + 1 < n_tiles:
                    nxt = []
                    for k2 in range(kb // 2):
                        wk = wpool.tile(
                            [128, 2, n_tile], in16, name=f"wt{n + 1}_{k2}", tag="wt"
                        )
                        nc.sync.dma_start(out=wk[:], in_=wt16_pair_src(n + 1, k2))
                        nxt.append(wk)
                    nxt8 = []
                    for j in range(qp):
                        wj = w8pool.tile(
                            [128, 2, n_tile], in8, name=f"wt8_{n + 1}_{j}", tag="w8"
                        )
                        nc.sync.dma_start(out=wj[:], in_=wt8_src(n + 1, j))
                        nxt8.append(wj)

                def wsl(k):  # fp16 rhs [128, n_tile] for k-tile of current n
                    if n == 0:
                        return wts[k][:]
                    return wts[k // 2][:, k % 2, :]

                def evict(ps_ap, m, col0, width):
                    o_sb = opool.tile(
                        [128, width], mybir.dt.float32, name="o_sb", tag="o_sb"
                    )
                    nc.vector.tensor_copy(o_sb[:], ps_ap)
                    nc.scalar.dma_start(
                        out=bass.AP(
                            out, m * 128 * f + n * n_tile + col0, [[f, 128], [1, width]]
                        ),
                        in_=o_sb[:],
                    )

                # Per n-group: run ALL fp16 matmuls for the 8 m-tiles first
                # (8 concurrent PSUM accumulation groups), then close every
                # group with one fp8-DoubleRow pass. This costs 2 PE dtype
                # switches per n-group instead of 2 per (m,n) group; a switch
                # measured ~38ns, so per-group interleaving would burn ~10us.
                last_n = n == n_tiles - 1
                m_full = m_tiles - 1 if (last_n and qp) else m_tiles
                pss = [
                    ppool.tile([128, n_tile], mybir.dt.float32, name=f"ps{n}_{m}", tag="ps")
                    for m in range(m_full)
                ]
                if n == 0:
                    # Ramp phase: k-outer / m-inner over the first chunk of k
                    # so each arriving (xt[k], wt[k]) pair immediately feeds
                    # m_tiles matmuls (PE starts as soon as the first pair
                    # lands).
                    k_half = min(3 * kb // 4, kb)
                    for k in range(k_half):
                        for m in range(m_full):
                            nc.tensor.matmul(
                                pss[m][:],
                                xsl(k, m),
                                wsl(k),
                                start=(k == 0),
                                stop=False,
                                skip_group_check=True,
                            )
                    for m in range(m_full):
                        for k in range(k_half, kb):
                            nc.tensor.matmul(
                                pss[m][:],
                                xsl(k, m),
                                wsl(k),
                                start=False,
                                stop=(k == kb - 1 and qp == 0),
                                skip_group_check=True,
                            )
                else:
                    for m in range(m_full):
                        for k in range(kb):
                            nc.tensor.matmul(
                                pss[m][:],
                                xsl(k, m),
                                wsl(k),
                                start=(k == 0),
                                stop=(k == kb - 1 and qp == 0),
                                skip_group_check=True,
                            )
                # fp8 DoubleRow pass + staggered evictions
                for m in range(m_full):
                    for j in range(qp):
                        nc.tensor.matmul(
                            pss[m][:],
                            x8sl(j, m),
                            wt8s[j][:],
                            start=False,
                            stop=(j == qp - 1),
                            perf_mode=DR,
                            skip_group_check=True,
                        )
                    evict(pss[m][:], m, 0, n_tile)
                if last_n and qp:
                    # The very last group is split column-wise (256,128,128)
                    # so the final eviction + output DMA (whose ~2us HBM
                    # write receipt is on the critical path) moves only 64KB
                    # after the last matmul; earlier chunks' drains overlap
                    # the later chunks' compute.
                    m = m_tiles - 1
                    for h, (c0, wd) in enumerate([(0, 256), (256, 128), (384, 128)]):
                        ps = ppool.tile(
                            [128, wd], mybir.dt.float32, name=f"psl{h}", tag="ps"
                        )
                        for k in range(kb):
                            nc.tensor.matmul(
                                ps[:],
                                xsl(k, m),
                                wsl(k)[:, c0 : c0 + wd],
                                start=(k == 0),
                                stop=False,
                                skip_group_check=True,
                            )
                        for j in range(qp):
                            nc.tensor.matmul(
                                ps[:],
                                x8sl(j, m),
                                wt8s[j][:, :, c0 : c0 + wd],
                                start=False,
                                stop=(j == qp - 1),
                                perf_mode=DR,
                                skip_group_check=True,
                            )
                        evict(ps[:], m, c0, wd)
                if n + 1 < n_tiles:
                    wts = nxt
                    wt8s = nxt8
    nc.compile()
    return nc


def _get_nc(q, base):
    if (q, base) not in _NC_CACHE:
        _NC_CACHE[(q, base)] = _build_nc(q, base)
    return _NC_CACHE[(q, base)]


def _soft_topk_scale(alpha):
    a = alpha.astype(np.float64)
    e = np.exp(a - a.max())
    return np.clip(KTOPK * (e / e.sum()), 0.0, 1.0).astype(np.float32)


def kernel(x, V, alpha):
    global _LAST_RESULTS
    from concourse.bass_utils import run_bass_kernel_spmd

    x = np.asarray(x, dtype=np.float32)
    V = np.asarray(V, dtype=np.float32)
    alpha = np.asarray(alpha, dtype=np.float32)

    q, s = Q, S
    kb = 32 - q
    np16 = ml_dtypes.bfloat16 if BASE == "bf16" else np.float16

    a_topk = _soft_topk_scale(alpha)
    VsT = np.ascontiguousarray((V * a_topk[:, None]).T)  # [c, p]
    W2 = np.concatenate([VsT, VsT], axis=1)  # [F, 2F] f32
    xT = np.ascontiguousarray(x.T)  # [F, B]

    max_off16 = F + 7 * 512 + ((kb - 1) * 128 + 127) * (2 * F - 1) + 511
    rows16 = max_off16 // (2 * F) + 1 if kb > 0 else 0
    W2_16 = W2[:rows16].astype(np16)
    xT16 = xT[: kb * 128].astype(np16)
    if q:
        W2_8 = (W2[kb * 128 :] * float(2**s)).astype(ml_dtypes.float8_e4m3)
        x8T = (xT[kb * 128 :] * float(2**-s)).astype(ml_dtypes.float8_e4m3)

    nc = _get_nc(q, BASE)
    in_maps = []
    for i in range(NCORES):
        m = {
            "xt": np.ascontiguousarray(xT16[:, i * BS : (i + 1) * BS]),
            "w2": W2_16,
        }
        if q:
            m["x8t"] = np.ascontiguousarray(x8T[:, i * BS : (i + 1) * BS])
            m["w28"] = W2_8
        in_maps.append(m)
    kwargs = {}
    if os.environ.get("GTOPK_TRACE"):
        try:
            import antenv.axon_hooks  # noqa: F401  (trace needs the hook)

            kwargs["trace"] = True
        except ImportError:
            pass
    res = run_bass_kernel_spmd(nc, in_maps, core_ids=list(range(NCORES)), **kwargs)
    _LAST_RESULTS = res
    return np.concatenate([r["out"] for r in res.results], axis=0)
